# revision 12
# baseline (speedup 1.0000x reference)
"""Trainium2 Bass kernel for the MemoryModule problem (v2).

Per batch element b (8 of them, one per NeuronCore):
    mk = memory_keys[:, b]  viewed as (Ck=128, M=8192)   [M = T*H*W]
    mv = memory_values[:, b] viewed as (Cv=512, M)
    qk = query_key[b]       viewed as (Ck=128, N=1024)   [N = H*W]
    S  = qk^T @ mk          (N, M)
    P  = softmax(S, axis=-1)
    mem = (P @ mv^T)^T      (Cv, N)
    out[b] = concat([query_value[b], mem], channel axis)

Device dataflow (v2 — derived from the measured v1 HW profile):
    - S^T tiles (128 m, 512 n) = matmul(lhsT=mk_tile, rhs=qk) in fp16.
    - exp on ScalarE (no max subtraction: |S| <~ 70 fits fp32/bf16), bf16 P^T.
    - PV: matmul(lhsT=P^T chunk, rhs=mv^T tile) bf16, accumulated over the
      64 m tiles in PSUM; n processed in halves of 512 (4 PV banks).
    - softmax denominator: VectorE accumulates P^T tiles into an SBUF
      partition-partial accumulator. v2 ships the raw accumulator and the
      UNNORMALIZED PV result to the host and divides there — this removes
      the ones-matmul partition reduction, reciprocal and on-device
      normalize from the critical tail, and frees the PSUM bank the
      denominator used, deepening the S^T pipeline to 4 banks.
    - DMA (v2): mv is host-packed so it loads in 8 x 1MB transfers
      (64KB per SDMA engine — near peak BW) instead of 64 x 128KB
      (8KB/engine, descriptor-dominated). mk (4 x 512KB) and mv chunks are
      interleaved in issue order so QK never starves while mv streams in.

PSUM: 4 S^T banks (quad-buffered) + 4 PV accumulators = 8.
"""

import os

import numpy as np
import ml_dtypes

T, B, Ck, Cv, H, W = 8, 8, 128, 512, 32, 32
HW = H * W            # 1024  (n dimension)
M = T * HW            # 8192  (memory / contraction dimension)
MT = M // 128         # 64 m-tiles
NQ = 2                # process n in halves
NQS = HW // NQ        # 512 columns of S^T per half
NCH = NQS // 128      # 4 PV accumulators per half
N_CORES = 8

# "f16": fp16 QK matmul (1 cyc/row on the PE, ~5e-4 input rounding)
QK_MODE = os.environ.get("KERNEL_QK_MODE", "f16")
# >1: repeat the full compute (incl. input DMAs) inside one NEFF via a
# hardware For_i loop, for HW timing via wall-clock deltas.
LOOP = int(os.environ.get("KERNEL_LOOP", "1"))
# timing diagnostics: "full" | "dma" | "compute" | "mmonly"
MODE = os.environ.get("KERNEL_MODE", "full")
# explicit ldweights before each matmul: ~16ns/MM faster in a pure-PE
# microbenchmark, but ~6.5us slower in the full pipeline (interleaved A/B;
# the extra instructions' semaphore waits clog the PE queue) — keep off
USE_LDW = os.environ.get("KERNEL_LDW", "0") == "1"
# how many tiles ahead of the PV consumer the exp (ACT) pipeline runs
EXP_AHEAD = int(os.environ.get("KERNEL_EXP_AHEAD", "2"))
# S^T PSUM pipeline depth (banks)
STP_BUFS = int(os.environ.get("KERNEL_STP_BUFS", "4"))
# 5-bank rotating PV accumulators (with STP_BUFS=3): the half's first PV
# chunk lands on the bank freed longest ago, removing the evacuation-WAR
# stall at each half boundary
PV5 = os.environ.get("KERNEL_PV5", "0") == "1"
# PSUM evacuation engine split: "mix" alternates DVE/ACT; "dve" puts all
# four copies on DVE (ACT's strict-FIFO queue holds the next half's exps,
# so ACT-side copies delay PV bank release by ~1.5us per half; DVE is
# ahead of PE at the boundary and releases banks in ~0.5us steps)
EVAC = os.environ.get("KERNEL_EVAC", "dve")
# batch the 4 per-half PV evacuations into one 1MB output DMA
BATCH_OUT = os.environ.get("KERNEL_BATCH_OUT", "0") == "1"
# compute passes per For_i trip in timed full mode (amortizes per-trip cost;
# 8 vs 4 measured -4.7us/pass median in paired interleaved A/B: the For_i
# back edge is an all-engine barrier, so the pipeline drain at each trip
# boundary is paid once per trip)
PASSES = int(os.environ.get("KERNEL_PASSES", "8"))
# pt (exp output) SBUF pool depth
PTP_BUFS = int(os.environ.get("KERNEL_PTP_BUFS", "6"))


def passes_per_iter(mode, loop):
    """Compute passes per For_i iteration (full mode double-buffers inputs
    across iterations with two body emissions per trip)."""
    return 2 if (mode == "full" and loop > 1) else 1

_CACHE = {}
LAST_RESULTS = None


def _build_nc(qk_mode, loop=1, mode="full"):
    import concourse.tile as tile
    import concourse.mybir as mybir
    from concourse import bacc

    f32 = mybir.dt.float32
    bf16 = mybir.dt.bfloat16
    f16 = mybir.dt.float16
    qk_dt = {"f16": f16, "f32r": f32, "f32": f32}[qk_mode]

    nc = bacc.Bacc()

    qk_d = nc.dram_tensor("qk", [Ck, HW], qk_dt, kind="ExternalInput")
    mk_d = nc.dram_tensor("mk", [Ck, M], qk_dt, kind="ExternalInput")
    # host-packed: mv_d[p, t*Cv + c] = mv[t*128 + p, c]
    mv_d = nc.dram_tensor("mv", [128, MT * Cv], bf16, kind="ExternalInput")
    num_d = nc.dram_tensor("num", [HW, Cv], f32, kind="ExternalOutput")
    acc_d = nc.dram_tensor("acc", [128, HW], f32, kind="ExternalOutput")

    Exp = mybir.ActivationFunctionType.Exp
    Copy = mybir.ActivationFunctionType.Copy
    AluOp = mybir.AluOpType

    MKC = 4               # mk DMA chunks
    MVC = 8               # mv DMA chunks

    def emit_dma(nc, tc, big, bset=0, nsfx=""):
        sfx = f"_{bset}"
        nm = sfx + nsfx
        qk_sb = big.tile([Ck, HW], qk_dt, tag="qk_sb" + sfx, name="qk_sb" + nm)
        nc.sync.dma_start(qk_sb[:, :NQS], qk_d[:, :NQS])
        nc.sync.dma_start(qk_sb[:, NQS:], qk_d[:, NQS:])
        mk_sb = big.tile([Ck, M], qk_dt, tag="mk_sb" + sfx, name="mk_sb" + nm)
        mv_sb = big.tile(
            [128, MT, Cv], bf16, tag="mv_sb" + sfx, name="mv_sb" + nm
        )
        # interleave mk / mv chunks so they drain in this order and QK can
        # start after the first mk chunk while mv streams in behind it
        mkw = M // MKC
        mvw = MT // MVC
        for i in range(MVC):
            if i < MKC:
                nc.sync.dma_start(
                    mk_sb[:, i * mkw : (i + 1) * mkw],
                    mk_d[:, i * mkw : (i + 1) * mkw],
                )
            nc.sync.dma_start(
                mv_sb[:, i * mvw : (i + 1) * mvw, :],
                mv_d[:, i * mvw * Cv : (i + 1) * mvw * Cv],
            )
        return qk_sb, mk_sb, mv_sb

    def body(nc, tc, big, ptp, accp, outp, stp, pvp, tiles, sfx="", pv_base=0):
        qk_sb, mk_sb, mv_sb = tiles

        class QState:
            def __init__(self, q):
                self.q = q
                self.sts = {}
                self.pts = {}

            def emit_st(self, m):
                st = stp.tile(
                    [128, NQS], f32, tag="st", name=f"st{sfx}_q{self.q}_m{m}"
                )
                wa = mk_sb[:, m * 128 : (m + 1) * 128]
                if USE_LDW:
                    nc.tensor.ldweights(wa)
                nc.tensor.matmul(
                    st[:],
                    wa,
                    qk_sb[:, self.q * NQS : (self.q + 1) * NQS],
                    start=True,
                    stop=True,
                )
                self.sts[m] = st

            def emit_exp(self, m):
                pt = ptp.tile(
                    [128, NQS], bf16, tag="pt", name=f"pt{sfx}_q{self.q}_m{m}"
                )
                nc.scalar.activation(pt[:], self.sts.pop(m)[:], Exp)
                self.pts[m] = pt

            def emit_acc(self, m):
                # VectorE: accumulate exp tiles for the softmax denominator
                if m == 0:
                    nc.vector.tensor_copy(self.acc[:], self.pts[m][:])
                else:
                    nc.vector.tensor_tensor(
                        self.acc[:], self.acc[:], self.pts[m][:], AluOp.add
                    )

            def prologue(self):
                q = self.q
                if PV5:
                    self.pv = [
                        pvp.tile(
                            [128, NQS], f32,
                            tag=f"pvb{(pv_base + q * NCH + i) % 5}",
                            name=f"pv{sfx}_q{q}_{i}",
                        )
                        for i in range(NCH)
                    ]
                else:
                    self.pv = [
                        pvp.tile(
                            [128, NQS], f32, tag=f"pv{i}", name=f"pv{sfx}_q{q}_{i}"
                        )
                        for i in range(NCH)
                    ]
                self.acc = accp.tile([128, NQS], f32, tag="acc", name=f"acc{sfx}_q{q}")
                # software pipeline: PE always has the next S^T ready, ACT
                # runs two tiles ahead of the PV consumers
                self.emit_st(0)
                self.emit_st(1)
                self.emit_exp(0)
                self.emit_st(2)
                self.emit_exp(1)
                self.emit_st(3)
                for e in range(2, EXP_AHEAD):
                    self.emit_exp(e)

            def mloop(self):
                for m in range(MT):
                    ptm = self.pts[m]
                    for nch in range(NCH):
                        wa = ptm[:, nch * 128 : (nch + 1) * 128]
                        if USE_LDW:
                            nc.tensor.ldweights(wa)
                        nc.tensor.matmul(
                            self.pv[nch][:],
                            wa,
                            mv_sb[:, m],
                            start=(m == 0),
                            stop=(m == MT - 1),
                        )
                    if m + EXP_AHEAD < MT:
                        self.emit_exp(m + EXP_AHEAD)
                    self.emit_acc(m)
                    del self.pts[m]
                    if m + 4 < MT:
                        self.emit_st(m + 4)

            def epilogue(self):
                q = self.q
                # raw partition-partial denominators out; host reduces
                nc.sync.dma_start(acc_d[:, q * NQS : (q + 1) * NQS], self.acc[:])
                if BATCH_OUT:
                    o = outp.tile(
                        [128, NCH, Cv], f32, tag="ob", name=f"ob{sfx}_q{q}"
                    )
                    for nch in range(NCH):
                        if nch % 2 == 0:
                            nc.vector.tensor_copy(o[:, nch], self.pv[nch][:])
                        else:
                            nc.scalar.activation(o[:, nch], self.pv[nch][:], Copy)
                    dst = num_d[q * NQS : (q + 1) * NQS, :].rearrange(
                        "(nch p) c -> p nch c", p=128
                    )
                    nc.sync.dma_start(dst, o[:])
                    return
                for nch in range(NCH):
                    o = outp.tile([128, Cv], f32, tag="o", name=f"o{sfx}_q{q}_{nch}")
                    # split PSUM evacuation across DVE and ACT. With PV5 the
                    # next half's chunks 1-3 reuse this half's chunk-0/1/2
                    # banks (chunk 3's bank is the next spare), so release
                    # 0-2 via DVE (ACT is busy with the next half's exps).
                    if EVAC == "dve":
                        use_dve = True
                    elif PV5:
                        use_dve = nch < 3
                    else:
                        use_dve = nch % 2 == 0
                    if use_dve:
                        nc.vector.tensor_copy(o[:], self.pv[nch][:])
                    else:
                        nc.scalar.activation(o[:], self.pv[nch][:], Copy)
                    n0 = q * NQS + nch * 128
                    nc.sync.dma_start(num_d[n0 : n0 + 128, :], o[:])

        # emit the next half's S^T prologue before this half's epilogue so
        # PE isn't FIFO-blocked behind the epilogue's dependencies
        interleave = os.environ.get("KERNEL_INTERLEAVE_Q", "1") == "1"
        states = [QState(q) for q in range(NQ)]
        states[0].prologue()
        for q in range(NQ):
            states[q].mloop()
            if interleave and q + 1 < NQ:
                states[q + 1].prologue()
            states[q].epilogue()
            if not interleave and q + 1 < NQ:
                states[q + 1].prologue()

    with tile.TileContext(nc) as tc:
        with (
            tc.tile_pool(name="big", bufs=1) as big,
            tc.tile_pool(name="ptp", bufs=PTP_BUFS) as ptp,
            tc.tile_pool(name="accp", bufs=3) as accp,
            tc.tile_pool(name="outp", bufs=(2 if BATCH_OUT else 6)) as outp,
            tc.tile_pool(name="stp", bufs=STP_BUFS, space="PSUM") as stp,
            tc.tile_pool(name="pvp", bufs=1, space="PSUM") as pvp,
        ):
            if mode == "mmonly" and loop > 1:
                # pure PE stream: same LDW+MM pair count/shapes as the real
                # kernel, but no ACT/DVE in the loop
                tiles = emit_dma(nc, tc, big)
                qk_sb, mk_sb, mv_sb = tiles
                dummy_pt = big.tile([128, NQS], mybir.dt.bfloat16, tag="dummy_pt", name="dummy_pt")
                nc.vector.memset(dummy_pt[:], 0.001)
                with tc.For_i(0, loop, 1):
                    for q in range(NQ):
                        pv = [
                            pvp.tile([128, NQS], f32, tag=f"pv{i}", name=f"mm_pv_q{q}_{i}")
                            for i in range(NCH)
                        ]
                        for m in range(MT):
                            st = stp.tile([128, NQS], f32, tag="st", name=f"mm_st_q{q}_m{m}")
                            nc.tensor.matmul(
                                st[:],
                                mk_sb[:, m * 128 : (m + 1) * 128],
                                qk_sb[:, q * NQS : (q + 1) * NQS],
                                start=True,
                                stop=True,
                            )
                            for nch in range(NCH):
                                nc.tensor.matmul(
                                    pv[nch][:],
                                    dummy_pt[:, nch * 128 : (nch + 1) * 128],
                                    mv_sb[:, m],
                                    start=(m == 0),
                                    stop=(m == MT - 1),
                                )
                        for nch in range(NCH):
                            o = outp.tile([128, Cv], f32, tag="o", name=f"mm_o_q{q}_{nch}")
                            nc.vector.tensor_copy(o[:], pv[nch][:])
                            nc.sync.dma_start(
                                num_d[(q * NCH + nch) * 128 : (q * NCH + nch + 1) * 128, :],
                                o[:],
                            )
            elif mode == "compute" and loop > 1:
                tiles = emit_dma(nc, tc, big)
                with tc.For_i(0, loop, 1):
                    body(nc, tc, big, ptp, accp, outp, stp, pvp, tiles)
            elif mode == "dma" and loop > 1:
                with tc.For_i(0, loop, 1):
                    emit_dma(nc, tc, big)
                    for j in range(8):
                        o = outp.tile([128, Cv], f32, tag="o", name=f"o_{j}")
                        nc.vector.memset(o[:], float(j))
                        nc.sync.dma_start(num_d[j * 128 : (j + 1) * 128, :], o[:])
            elif loop > 1:
                # full mode, timed: PASSES compute passes per trip over two
                # alternating input buffer sets, so input DMAs stream
                # underneath compute. Back-edge WAR: next trip's set-k DMAs
                # only wait on this trip's second-to-last readers of set k,
                # so they overlap the final passes. Single-buffered, the WAR
                # hazard serializes DMA after compute (~36us exposed/pass).
                with tc.For_i(0, max(loop // PASSES, 1), 1):
                    tiles = {0: emit_dma(nc, tc, big, 0)}
                    if PASSES > 1:
                        tiles[1] = emit_dma(nc, tc, big, 1)
                    for p in range(PASSES):
                        body(
                            nc, tc, big, ptp, accp, outp, stp, pvp,
                            tiles[p], f"p{p}",
                            pv_base=(p * NQ * NCH) % 5 if PV5 else 0,
                        )
                        if p + 2 < PASSES:
                            tiles[p + 2] = emit_dma(
                                nc, tc, big, p % 2, f"_p{p + 2}"
                            )
            else:
                tiles = emit_dma(nc, tc, big)
                body(nc, tc, big, ptp, accp, outp, stp, pvp, tiles)

    nc.finalize()
    return nc


class _null:
    def __enter__(self):
        return None

    def __exit__(self, *a):
        return False


def _get_nc():
    key = ("nc", QK_MODE, LOOP, MODE)
    if key not in _CACHE:
        _CACHE[key] = _build_nc(QK_MODE, LOOP, MODE)
    return _CACHE[key]


def _prep_core_inputs(memory_keys, memory_values, query_key, b):
    np_qk_dt = np.float16 if QK_MODE == "f16" else np.float32
    qk = query_key[b].reshape(Ck, HW).astype(np_qk_dt)
    mk = memory_keys[:, b].transpose(1, 0, 2, 3).astype(np_qk_dt).reshape(Ck, M)
    # (M, Cv) -> tiles of 128 m-rows, partition-major: [128, MT*Cv]
    mv = (
        memory_values[:, b]
        .transpose(0, 2, 3, 1)
        .astype(ml_dtypes.bfloat16)
        .reshape(MT, 128, Cv)
        .transpose(1, 0, 2)
        .reshape(128, MT * Cv)
    )
    mv = np.ascontiguousarray(mv)
    return {"qk": qk, "mk": mk, "mv": mv}


_RUNNER = {}


def _get_runner():
    """Build the sharded PJRT callable once and reuse it."""
    if "r" not in _RUNNER:
        import jax
        from jax.sharding import Mesh, PartitionSpec, NamedSharding
        from jax.experimental.shard_map import shard_map

        import concourse.mybir as mybir
        from concourse import bass2jax
        from concourse.bass2jax import _bass_exec_p, install_neuronx_cc_hook

        nc = _get_nc()
        install_neuronx_cc_hook()
        pname = nc.partition_id_tensor.name if nc.partition_id_tensor else None
        in_names, out_names, out_avals = [], [], []
        for alloc in nc.m.functions[0].allocations:
            if not isinstance(alloc, mybir.MemoryLocationSet):
                continue
            name = alloc.memorylocations[0].name
            if alloc.kind == "ExternalInput":
                if name != pname:
                    in_names.append(name)
            elif alloc.kind == "ExternalOutput":
                out_names.append(name)
                out_avals.append(
                    jax.core.ShapedArray(
                        tuple(alloc.tensor_shape), mybir.dt.np(alloc.dtype)
                    )
                )
        n_params = len(in_names)
        all_in = list(in_names) + list(out_names) + ([pname] if pname else [])

        def _body(*args):
            operands = list(args)
            if pname is not None:
                operands.append(bass2jax.partition_id_tensor())
            return tuple(
                _bass_exec_p.bind(
                    *operands,
                    out_avals=tuple(out_avals),
                    in_names=tuple(all_in),
                    out_names=tuple(out_names),
                    lowering_input_output_aliases=(),
                    sim_require_finite=True,
                    sim_require_nnan=True,
                    nc=nc,
                )
            )

        mesh = Mesh(np.asarray(jax.devices()[:N_CORES]), ("core",))
        n_outs = len(out_names)
        sharded = jax.jit(
            shard_map(
                _body,
                mesh=mesh,
                in_specs=(PartitionSpec("core"),) * (n_params + n_outs),
                out_specs=(PartitionSpec("core"),) * n_outs,
                check_rep=False,
            ),
            keep_unused=True,
        )
        sh = NamedSharding(mesh, PartitionSpec("core"))
        zeros = [
            jax.device_put(
                np.zeros((N_CORES * a.shape[0], *a.shape[1:]), a.dtype), sh
            )
            for a in out_avals
        ]
        _RUNNER["r"] = (sharded, sh, in_names, out_names, zeros)
    return _RUNNER["r"]


def _finish(num_full, acc_full, query_value):
    """Host-side softmax normalization + output assembly."""
    mems = []
    for bb in range(N_CORES):
        den = acc_full[bb].astype(np.float64).sum(axis=0)  # (HW,)
        mem = (num_full[bb].astype(np.float64) / den[:, None]).T  # (Cv, HW)
        mems.append(mem.astype(np.float32).reshape(Cv, H, W))
    return np.concatenate([query_value, np.stack(mems)], axis=1)


def kernel(memory_keys, memory_values, query_key, query_value):
    global LAST_RESULTS
    memory_keys = np.asarray(memory_keys, dtype=np.float32)
    memory_values = np.asarray(memory_values, dtype=np.float32)
    query_key = np.asarray(query_key, dtype=np.float32)
    query_value = np.asarray(query_value, dtype=np.float32)

    in_maps = [
        _prep_core_inputs(memory_keys, memory_values, query_key, b)
        for b in range(N_CORES)
    ]
    try:
        import jax

        sharded, sh, in_names, out_names, zeros = _get_runner()
        dev_in = [
            jax.device_put(
                np.concatenate([in_maps[c][n] for c in range(N_CORES)], 0), sh
            )
            for n in in_names
        ]
        outs = sharded(*dev_in, *zeros)
        res = {n: np.asarray(o) for n, o in zip(out_names, outs)}
        num_full = res["num"].reshape(N_CORES, HW, Cv)
        acc_full = res["acc"].reshape(N_CORES, 128, HW)
    except Exception:
        from concourse.bass_utils import run_bass_kernel_spmd

        res = run_bass_kernel_spmd(
            _get_nc(), in_maps, core_ids=list(range(N_CORES))
        )
        LAST_RESULTS = res
        num_full = np.stack([res.results[b]["num"] for b in range(N_CORES)])
        acc_full = np.stack([res.results[b]["acc"] for b in range(N_CORES)])

    return _finish(num_full, acc_full, query_value)



# revision 16
# speedup vs baseline: 1.1221x; 1.1221x over previous
"""Trainium2 Bass kernel for the MemoryModule problem (v2).

Per batch element b (8 of them, one per NeuronCore):
    mk = memory_keys[:, b]  viewed as (Ck=128, M=8192)   [M = T*H*W]
    mv = memory_values[:, b] viewed as (Cv=512, M)
    qk = query_key[b]       viewed as (Ck=128, N=1024)   [N = H*W]
    S  = qk^T @ mk          (N, M)
    P  = softmax(S, axis=-1)
    mem = (P @ mv^T)^T      (Cv, N)
    out[b] = concat([query_value[b], mem], channel axis)

Device dataflow (v2 — derived from the measured v1 HW profile):
    - S^T tiles (128 m, 512 n) = matmul(lhsT=mk_tile, rhs=qk) in fp16.
    - exp on ScalarE (no max subtraction: |S| <~ 70 fits fp32/bf16), bf16 P^T.
    - PV: matmul(lhsT=P^T chunk, rhs=mv^T tile) bf16, accumulated over the
      64 m tiles in PSUM; n processed in halves of 512 (4 PV banks).
    - softmax denominator: VectorE accumulates P^T tiles into an SBUF
      partition-partial accumulator. v2 ships the raw accumulator and the
      UNNORMALIZED PV result to the host and divides there — this removes
      the ones-matmul partition reduction, reciprocal and on-device
      normalize from the critical tail, and frees the PSUM bank the
      denominator used, deepening the S^T pipeline to 4 banks.
    - DMA (v2): mv is host-packed so it loads in 8 x 1MB transfers
      (64KB per SDMA engine — near peak BW) instead of 64 x 128KB
      (8KB/engine, descriptor-dominated). mk (4 x 512KB) and mv chunks are
      interleaved in issue order so QK never starves while mv streams in.

PSUM: 4 S^T banks (quad-buffered) + 4 PV accumulators = 8.
"""

import os

import numpy as np
import ml_dtypes

T, B, Ck, Cv, H, W = 8, 8, 128, 512, 32, 32
HW = H * W            # 1024  (n dimension)
M = T * HW            # 8192  (memory / contraction dimension)
MT = M // 128         # 64 m-tiles
NQ = 2                # process n in halves
NQS = HW // NQ        # 512 columns of S^T per half
NCH = NQS // 128      # 4 PV accumulators per half
N_CORES = 8

# "f16": fp16 QK matmul (1 cyc/row on the PE, ~5e-4 input rounding)
QK_MODE = os.environ.get("KERNEL_QK_MODE", "f16")
# >1: repeat the full compute (incl. input DMAs) inside one NEFF via a
# hardware For_i loop, for HW timing via wall-clock deltas.
LOOP = int(os.environ.get("KERNEL_LOOP", "1"))
# timing diagnostics: "full" | "dma" | "compute" | "mmonly"
MODE = os.environ.get("KERNEL_MODE", "full")
# explicit ldweights before each matmul: ~16ns/MM faster in a pure-PE
# microbenchmark, but ~6.5us slower in the full pipeline (interleaved A/B;
# the extra instructions' semaphore waits clog the PE queue) — keep off
USE_LDW = os.environ.get("KERNEL_LDW", "0") == "1"
# how many tiles ahead of the PV consumer the exp (ACT) pipeline runs
EXP_AHEAD = int(os.environ.get("KERNEL_EXP_AHEAD", "2"))
# S^T PSUM pipeline depth (banks)
STP_BUFS = int(os.environ.get("KERNEL_STP_BUFS", "4"))
# 5-bank rotating PV accumulators (with STP_BUFS=3): the half's first PV
# chunk lands on the bank freed longest ago, removing the evacuation-WAR
# stall at each half boundary
PV5 = os.environ.get("KERNEL_PV5", "0") == "1"
# PSUM evacuation engine split: "mix" alternates DVE/ACT; "dve" puts all
# four copies on DVE (ACT's strict-FIFO queue holds the next half's exps,
# so ACT-side copies delay PV bank release by ~1.5us per half; DVE is
# ahead of PE at the boundary and releases banks in ~0.5us steps)
EVAC = os.environ.get("KERNEL_EVAC", "dve")
# defer each pass's final epilogue past the next pass's S^T prologue
# (cross-pass analog of INTERLEAVE_Q)
XPASS = os.environ.get("KERNEL_XPASS", "1") == "1"
# batch the 4 per-half PV evacuations into one 1MB output DMA
BATCH_OUT = os.environ.get("KERNEL_BATCH_OUT", "0") == "1"
# compute passes per For_i trip in timed full mode (amortizes per-trip cost;
# 8 vs 4 measured -4.7us/pass median in paired interleaved A/B: the For_i
# back edge is an all-engine barrier, so the pipeline drain at each trip
# boundary is paid once per trip)
PASSES = int(os.environ.get("KERNEL_PASSES", "8"))
# pt (exp output) SBUF pool depth
PTP_BUFS = int(os.environ.get("KERNEL_PTP_BUFS", "6"))


def passes_per_iter(mode, loop):
    """Compute passes per For_i iteration (full mode double-buffers inputs
    across iterations with two body emissions per trip)."""
    return 2 if (mode == "full" and loop > 1) else 1

_CACHE = {}
LAST_RESULTS = None


def _build_nc(qk_mode, loop=1, mode="full"):
    import concourse.tile as tile
    import concourse.mybir as mybir
    from concourse import bacc

    f32 = mybir.dt.float32
    bf16 = mybir.dt.bfloat16
    f16 = mybir.dt.float16
    qk_dt = {"f16": f16, "f32r": f32, "f32": f32}[qk_mode]

    nc = bacc.Bacc()

    qk_d = nc.dram_tensor("qk", [Ck, HW], qk_dt, kind="ExternalInput")
    mk_d = nc.dram_tensor("mk", [Ck, M], qk_dt, kind="ExternalInput")
    # host-packed: mv_d[p, t*Cv + c] = mv[t*128 + p, c]
    mv_d = nc.dram_tensor("mv", [128, MT * Cv], bf16, kind="ExternalInput")
    num_d = nc.dram_tensor("num", [HW, Cv], f32, kind="ExternalOutput")
    acc_d = nc.dram_tensor("acc", [128, HW], f32, kind="ExternalOutput")

    Exp = mybir.ActivationFunctionType.Exp
    Copy = mybir.ActivationFunctionType.Copy
    AluOp = mybir.AluOpType

    MKC = 4               # mk DMA chunks
    MVC = 8               # mv DMA chunks

    def emit_dma(nc, tc, big, bset=0, nsfx=""):
        sfx = f"_{bset}"
        nm = sfx + nsfx
        qk_sb = big.tile([Ck, HW], qk_dt, tag="qk_sb" + sfx, name="qk_sb" + nm)
        nc.sync.dma_start(qk_sb[:, :NQS], qk_d[:, :NQS])
        nc.sync.dma_start(qk_sb[:, NQS:], qk_d[:, NQS:])
        mk_sb = big.tile([Ck, M], qk_dt, tag="mk_sb" + sfx, name="mk_sb" + nm)
        mv_sb = big.tile(
            [128, MT, Cv], bf16, tag="mv_sb" + sfx, name="mv_sb" + nm
        )
        # interleave mk / mv chunks so they drain in this order and QK can
        # start after the first mk chunk while mv streams in behind it
        mkw = M // MKC
        mvw = MT // MVC
        for i in range(MVC):
            if i < MKC:
                nc.sync.dma_start(
                    mk_sb[:, i * mkw : (i + 1) * mkw],
                    mk_d[:, i * mkw : (i + 1) * mkw],
                )
            nc.sync.dma_start(
                mv_sb[:, i * mvw : (i + 1) * mvw, :],
                mv_d[:, i * mvw * Cv : (i + 1) * mvw * Cv],
            )
        return qk_sb, mk_sb, mv_sb

    def body(
        nc, tc, big, ptp, accp, outp, stp, pvp, tiles, sfx="", pv_base=0,
        pending=None, defer=False,
    ):
        """Emit one compute pass. `pending` is the previous pass's final
        half-state whose epilogue was deferred; it is emitted after this
        pass's first S^T prologue so the PE starts the new pass's QK MMs
        while DVE/DMA drain the old epilogue. With `defer`, this pass's
        own final epilogue is likewise left to the next body (the caller
        must close the last one before the For_i back edge)."""
        qk_sb, mk_sb, mv_sb = tiles

        class QState:
            def __init__(self, q):
                self.q = q
                self.sts = {}
                self.pts = {}

            def emit_st(self, m):
                st = stp.tile(
                    [128, NQS], f32, tag="st", name=f"st{sfx}_q{self.q}_m{m}"
                )
                wa = mk_sb[:, m * 128 : (m + 1) * 128]
                if USE_LDW:
                    nc.tensor.ldweights(wa)
                nc.tensor.matmul(
                    st[:],
                    wa,
                    qk_sb[:, self.q * NQS : (self.q + 1) * NQS],
                    start=True,
                    stop=True,
                )
                self.sts[m] = st

            def emit_exp(self, m):
                pt = ptp.tile(
                    [128, NQS], bf16, tag="pt", name=f"pt{sfx}_q{self.q}_m{m}"
                )
                nc.scalar.activation(pt[:], self.sts.pop(m)[:], Exp)
                self.pts[m] = pt

            def emit_acc(self, m):
                # VectorE: accumulate exp tiles for the softmax denominator
                if m == 0:
                    nc.vector.tensor_copy(self.acc[:], self.pts[m][:])
                else:
                    nc.vector.tensor_tensor(
                        self.acc[:], self.acc[:], self.pts[m][:], AluOp.add
                    )

            def prologue(self):
                q = self.q
                if PV5:
                    self.pv = [
                        pvp.tile(
                            [128, NQS], f32,
                            tag=f"pvb{(pv_base + q * NCH + i) % 5}",
                            name=f"pv{sfx}_q{q}_{i}",
                        )
                        for i in range(NCH)
                    ]
                else:
                    self.pv = [
                        pvp.tile(
                            [128, NQS], f32, tag=f"pv{i}", name=f"pv{sfx}_q{q}_{i}"
                        )
                        for i in range(NCH)
                    ]
                self.acc = accp.tile([128, NQS], f32, tag="acc", name=f"acc{sfx}_q{q}")
                # software pipeline: PE always has the next S^T ready, ACT
                # runs two tiles ahead of the PV consumers
                self.emit_st(0)
                self.emit_st(1)
                self.emit_exp(0)
                self.emit_st(2)
                self.emit_exp(1)
                self.emit_st(3)
                for e in range(2, EXP_AHEAD):
                    self.emit_exp(e)

            def mloop(self):
                for m in range(MT):
                    ptm = self.pts[m]
                    for nch in range(NCH):
                        wa = ptm[:, nch * 128 : (nch + 1) * 128]
                        if USE_LDW:
                            nc.tensor.ldweights(wa)
                        nc.tensor.matmul(
                            self.pv[nch][:],
                            wa,
                            mv_sb[:, m],
                            start=(m == 0),
                            stop=(m == MT - 1),
                        )
                    if m + EXP_AHEAD < MT:
                        self.emit_exp(m + EXP_AHEAD)
                    self.emit_acc(m)
                    del self.pts[m]
                    if m + 4 < MT:
                        self.emit_st(m + 4)

            def epilogue(self):
                q = self.q
                # raw partition-partial denominators out; host reduces
                nc.sync.dma_start(acc_d[:, q * NQS : (q + 1) * NQS], self.acc[:])
                if BATCH_OUT:
                    o = outp.tile(
                        [128, NCH, Cv], f32, tag="ob", name=f"ob{sfx}_q{q}"
                    )
                    for nch in range(NCH):
                        if nch % 2 == 0:
                            nc.vector.tensor_copy(o[:, nch], self.pv[nch][:])
                        else:
                            nc.scalar.activation(o[:, nch], self.pv[nch][:], Copy)
                    dst = num_d[q * NQS : (q + 1) * NQS, :].rearrange(
                        "(nch p) c -> p nch c", p=128
                    )
                    nc.sync.dma_start(dst, o[:])
                    return
                for nch in range(NCH):
                    o = outp.tile([128, Cv], f32, tag="o", name=f"o{sfx}_q{q}_{nch}")
                    # split PSUM evacuation across DVE and ACT. With PV5 the
                    # next half's chunks 1-3 reuse this half's chunk-0/1/2
                    # banks (chunk 3's bank is the next spare), so release
                    # 0-2 via DVE (ACT is busy with the next half's exps).
                    if EVAC == "dve":
                        use_dve = True
                    elif PV5:
                        use_dve = nch < 3
                    else:
                        use_dve = nch % 2 == 0
                    if use_dve:
                        nc.vector.tensor_copy(o[:], self.pv[nch][:])
                    else:
                        nc.scalar.activation(o[:], self.pv[nch][:], Copy)
                    n0 = q * NQS + nch * 128
                    nc.sync.dma_start(num_d[n0 : n0 + 128, :], o[:])

        # emit the next half's S^T prologue before this half's epilogue so
        # PE isn't FIFO-blocked behind the epilogue's dependencies
        interleave = os.environ.get("KERNEL_INTERLEAVE_Q", "1") == "1"
        states = [QState(q) for q in range(NQ)]
        states[0].prologue()
        if pending is not None:
            pending.epilogue()
        for q in range(NQ):
            states[q].mloop()
            if q + 1 < NQ:
                if interleave:
                    states[q + 1].prologue()
                    states[q].epilogue()
                else:
                    states[q].epilogue()
                    states[q + 1].prologue()
            elif defer and interleave:
                return states[q]
            else:
                states[q].epilogue()
        return None

    with tile.TileContext(nc) as tc:
        with (
            tc.tile_pool(name="big", bufs=1) as big,
            tc.tile_pool(name="ptp", bufs=PTP_BUFS) as ptp,
            tc.tile_pool(name="accp", bufs=3) as accp,
            tc.tile_pool(name="outp", bufs=(2 if BATCH_OUT else 6)) as outp,
            tc.tile_pool(name="stp", bufs=STP_BUFS, space="PSUM") as stp,
            tc.tile_pool(name="pvp", bufs=1, space="PSUM") as pvp,
        ):
            if mode == "mmonly" and loop > 1:
                # pure PE stream: same LDW+MM pair count/shapes as the real
                # kernel, but no ACT/DVE in the loop
                tiles = emit_dma(nc, tc, big)
                qk_sb, mk_sb, mv_sb = tiles
                dummy_pt = big.tile([128, NQS], mybir.dt.bfloat16, tag="dummy_pt", name="dummy_pt")
                nc.vector.memset(dummy_pt[:], 0.001)
                with tc.For_i(0, loop, 1):
                    for q in range(NQ):
                        pv = [
                            pvp.tile([128, NQS], f32, tag=f"pv{i}", name=f"mm_pv_q{q}_{i}")
                            for i in range(NCH)
                        ]
                        for m in range(MT):
                            st = stp.tile([128, NQS], f32, tag="st", name=f"mm_st_q{q}_m{m}")
                            nc.tensor.matmul(
                                st[:],
                                mk_sb[:, m * 128 : (m + 1) * 128],
                                qk_sb[:, q * NQS : (q + 1) * NQS],
                                start=True,
                                stop=True,
                            )
                            for nch in range(NCH):
                                nc.tensor.matmul(
                                    pv[nch][:],
                                    dummy_pt[:, nch * 128 : (nch + 1) * 128],
                                    mv_sb[:, m],
                                    start=(m == 0),
                                    stop=(m == MT - 1),
                                )
                        for nch in range(NCH):
                            o = outp.tile([128, Cv], f32, tag="o", name=f"mm_o_q{q}_{nch}")
                            nc.vector.tensor_copy(o[:], pv[nch][:])
                            nc.sync.dma_start(
                                num_d[(q * NCH + nch) * 128 : (q * NCH + nch + 1) * 128, :],
                                o[:],
                            )
            elif mode == "compute" and loop > 1:
                tiles = emit_dma(nc, tc, big)
                with tc.For_i(0, loop, 1):
                    body(nc, tc, big, ptp, accp, outp, stp, pvp, tiles)
            elif mode == "dma" and loop > 1:
                with tc.For_i(0, loop, 1):
                    emit_dma(nc, tc, big)
                    for j in range(8):
                        o = outp.tile([128, Cv], f32, tag="o", name=f"o_{j}")
                        nc.vector.memset(o[:], float(j))
                        nc.sync.dma_start(num_d[j * 128 : (j + 1) * 128, :], o[:])
            elif loop > 1:
                # full mode, timed: PASSES compute passes per trip over two
                # alternating input buffer sets, so input DMAs stream
                # underneath compute. Back-edge WAR: next trip's set-k DMAs
                # only wait on this trip's second-to-last readers of set k,
                # so they overlap the final passes. Single-buffered, the WAR
                # hazard serializes DMA after compute (~36us exposed/pass).
                with tc.For_i(0, max(loop // PASSES, 1), 1):
                    tiles = {0: emit_dma(nc, tc, big, 0)}
                    if PASSES > 1:
                        tiles[1] = emit_dma(nc, tc, big, 1)
                    pend = None
                    for p in range(PASSES):
                        pend = body(
                            nc, tc, big, ptp, accp, outp, stp, pvp,
                            tiles[p], f"p{p}",
                            pv_base=(p * NQ * NCH) % 5 if PV5 else 0,
                            pending=pend,
                            defer=XPASS and (p + 1 < PASSES),
                        )
                        if p + 2 < PASSES:
                            tiles[p + 2] = emit_dma(
                                nc, tc, big, p % 2, f"_p{p + 2}"
                            )
                    assert pend is None
            else:
                tiles = emit_dma(nc, tc, big)
                body(nc, tc, big, ptp, accp, outp, stp, pvp, tiles)

    nc.finalize()
    return nc


class _null:
    def __enter__(self):
        return None

    def __exit__(self, *a):
        return False


def _get_nc():
    key = ("nc", QK_MODE, LOOP, MODE)
    if key not in _CACHE:
        _CACHE[key] = _build_nc(QK_MODE, LOOP, MODE)
    return _CACHE[key]


def _prep_core_inputs(memory_keys, memory_values, query_key, b):
    np_qk_dt = np.float16 if QK_MODE == "f16" else np.float32
    qk = query_key[b].reshape(Ck, HW).astype(np_qk_dt)
    mk = memory_keys[:, b].transpose(1, 0, 2, 3).astype(np_qk_dt).reshape(Ck, M)
    # (M, Cv) -> tiles of 128 m-rows, partition-major: [128, MT*Cv]
    mv = (
        memory_values[:, b]
        .transpose(0, 2, 3, 1)
        .astype(ml_dtypes.bfloat16)
        .reshape(MT, 128, Cv)
        .transpose(1, 0, 2)
        .reshape(128, MT * Cv)
    )
    mv = np.ascontiguousarray(mv)
    return {"qk": qk, "mk": mk, "mv": mv}


_RUNNER = {}


def _get_runner():
    """Build the sharded PJRT callable once and reuse it."""
    if "r" not in _RUNNER:
        import jax
        from jax.sharding import Mesh, PartitionSpec, NamedSharding
        from jax.experimental.shard_map import shard_map

        import concourse.mybir as mybir
        from concourse import bass2jax
        from concourse.bass2jax import _bass_exec_p, install_neuronx_cc_hook

        nc = _get_nc()
        install_neuronx_cc_hook()
        pname = nc.partition_id_tensor.name if nc.partition_id_tensor else None
        in_names, out_names, out_avals = [], [], []
        for alloc in nc.m.functions[0].allocations:
            if not isinstance(alloc, mybir.MemoryLocationSet):
                continue
            name = alloc.memorylocations[0].name
            if alloc.kind == "ExternalInput":
                if name != pname:
                    in_names.append(name)
            elif alloc.kind == "ExternalOutput":
                out_names.append(name)
                out_avals.append(
                    jax.core.ShapedArray(
                        tuple(alloc.tensor_shape), mybir.dt.np(alloc.dtype)
                    )
                )
        n_params = len(in_names)
        all_in = list(in_names) + list(out_names) + ([pname] if pname else [])

        def _body(*args):
            operands = list(args)
            if pname is not None:
                operands.append(bass2jax.partition_id_tensor())
            return tuple(
                _bass_exec_p.bind(
                    *operands,
                    out_avals=tuple(out_avals),
                    in_names=tuple(all_in),
                    out_names=tuple(out_names),
                    lowering_input_output_aliases=(),
                    sim_require_finite=True,
                    sim_require_nnan=True,
                    nc=nc,
                )
            )

        mesh = Mesh(np.asarray(jax.devices()[:N_CORES]), ("core",))
        n_outs = len(out_names)
        sharded = jax.jit(
            shard_map(
                _body,
                mesh=mesh,
                in_specs=(PartitionSpec("core"),) * (n_params + n_outs),
                out_specs=(PartitionSpec("core"),) * n_outs,
                check_rep=False,
            ),
            keep_unused=True,
        )
        sh = NamedSharding(mesh, PartitionSpec("core"))
        zeros = [
            jax.device_put(
                np.zeros((N_CORES * a.shape[0], *a.shape[1:]), a.dtype), sh
            )
            for a in out_avals
        ]
        _RUNNER["r"] = (sharded, sh, in_names, out_names, zeros)
    return _RUNNER["r"]


def _finish(num_full, acc_full, query_value):
    """Host-side softmax normalization + output assembly."""
    mems = []
    for bb in range(N_CORES):
        den = acc_full[bb].astype(np.float64).sum(axis=0)  # (HW,)
        mem = (num_full[bb].astype(np.float64) / den[:, None]).T  # (Cv, HW)
        mems.append(mem.astype(np.float32).reshape(Cv, H, W))
    return np.concatenate([query_value, np.stack(mems)], axis=1)


def kernel(memory_keys, memory_values, query_key, query_value):
    global LAST_RESULTS
    memory_keys = np.asarray(memory_keys, dtype=np.float32)
    memory_values = np.asarray(memory_values, dtype=np.float32)
    query_key = np.asarray(query_key, dtype=np.float32)
    query_value = np.asarray(query_value, dtype=np.float32)

    in_maps = [
        _prep_core_inputs(memory_keys, memory_values, query_key, b)
        for b in range(N_CORES)
    ]
    try:
        import jax

        sharded, sh, in_names, out_names, zeros = _get_runner()
        dev_in = [
            jax.device_put(
                np.concatenate([in_maps[c][n] for c in range(N_CORES)], 0), sh
            )
            for n in in_names
        ]
        outs = sharded(*dev_in, *zeros)
        res = {n: np.asarray(o) for n, o in zip(out_names, outs)}
        num_full = res["num"].reshape(N_CORES, HW, Cv)
        acc_full = res["acc"].reshape(N_CORES, 128, HW)
    except Exception:
        from concourse.bass_utils import run_bass_kernel_spmd

        res = run_bass_kernel_spmd(
            _get_nc(), in_maps, core_ids=list(range(N_CORES))
        )
        LAST_RESULTS = res
        num_full = np.stack([res.results[b]["num"] for b in range(N_CORES)])
        acc_full = np.stack([res.results[b]["acc"] for b in range(N_CORES)])

    return _finish(num_full, acc_full, query_value)



# revision 18
# speedup vs baseline: 1.1326x; 1.0093x over previous
"""Trainium2 Bass kernel for the MemoryModule problem (v3).

v3 changes over v2 (each validated by paired interleaved A/B, which is
mandatory on this setup — run-to-run drift reaches 1.7x):
    - PASSES 4 -> 8: the For_i back edge is an all-engine barrier, so each
      trip pays a full pipeline drain; 8 bodies/trip halves its amortized
      cost (-4.7us/pass median paired).
    - XPASS: each pass's final epilogue is deferred past the NEXT pass's
      S^T prologue (cross-pass analog of the existing half-interleave), so
      the PE starts the next pass's QK matmuls while DVE/DMA drain the old
      epilogue (-4.5us/pass median paired, 16/20 rounds negative).
    - EVAC=dve: all four PV-bank evacuations on VectorE. ScalarE's
      strict-FIFO queue holds the next half's exps, so ACT-side copies
      delayed PV bank release by ~1.5us per half boundary.
    - Buried avenues (measured, do not revisit): explicit ldweights and
      weight reuse are SLOWER than implicit per-MM loads (LDW fully
      overlaps); PSUM bank switching is free (chain16 == accum); PV5
      (5-bank PV rotation + STP_BUFS=3) is ~4us WORSE; fp8/DoubleRow fails
      the softmax range/accuracy budget. The N=512 MM stream floor is
      ~262ns/MM (~0.51ns/col, effective PE clock ~1.95GHz, not 2.4) giving
      a ~168us/pass PE roofline; this kernel measures ~170us in a clean
      machine window.
"""

"""v2 design notes below.

Per batch element b (8 of them, one per NeuronCore):
    mk = memory_keys[:, b]  viewed as (Ck=128, M=8192)   [M = T*H*W]
    mv = memory_values[:, b] viewed as (Cv=512, M)
    qk = query_key[b]       viewed as (Ck=128, N=1024)   [N = H*W]
    S  = qk^T @ mk          (N, M)
    P  = softmax(S, axis=-1)
    mem = (P @ mv^T)^T      (Cv, N)
    out[b] = concat([query_value[b], mem], channel axis)

Device dataflow (v2 — derived from the measured v1 HW profile):
    - S^T tiles (128 m, 512 n) = matmul(lhsT=mk_tile, rhs=qk) in fp16.
    - exp on ScalarE (no max subtraction: |S| <~ 70 fits fp32/bf16), bf16 P^T.
    - PV: matmul(lhsT=P^T chunk, rhs=mv^T tile) bf16, accumulated over the
      64 m tiles in PSUM; n processed in halves of 512 (4 PV banks).
    - softmax denominator: VectorE accumulates P^T tiles into an SBUF
      partition-partial accumulator. v2 ships the raw accumulator and the
      UNNORMALIZED PV result to the host and divides there — this removes
      the ones-matmul partition reduction, reciprocal and on-device
      normalize from the critical tail, and frees the PSUM bank the
      denominator used, deepening the S^T pipeline to 4 banks.
    - DMA (v2): mv is host-packed so it loads in 8 x 1MB transfers
      (64KB per SDMA engine — near peak BW) instead of 64 x 128KB
      (8KB/engine, descriptor-dominated). mk (4 x 512KB) and mv chunks are
      interleaved in issue order so QK never starves while mv streams in.

PSUM: 4 S^T banks (quad-buffered) + 4 PV accumulators = 8.
"""

import os

import numpy as np
import ml_dtypes

T, B, Ck, Cv, H, W = 8, 8, 128, 512, 32, 32
HW = H * W            # 1024  (n dimension)
M = T * HW            # 8192  (memory / contraction dimension)
MT = M // 128         # 64 m-tiles
NQ = 2                # process n in halves
NQS = HW // NQ        # 512 columns of S^T per half
NCH = NQS // 128      # 4 PV accumulators per half
N_CORES = 8

# "f16": fp16 QK matmul (1 cyc/row on the PE, ~5e-4 input rounding)
QK_MODE = os.environ.get("KERNEL_QK_MODE", "f16")
# >1: repeat the full compute (incl. input DMAs) inside one NEFF via a
# hardware For_i loop, for HW timing via wall-clock deltas.
LOOP = int(os.environ.get("KERNEL_LOOP", "1"))
# timing diagnostics: "full" | "dma" | "compute" | "mmonly"
MODE = os.environ.get("KERNEL_MODE", "full")
# explicit ldweights before each matmul: ~16ns/MM faster in a pure-PE
# microbenchmark, but ~6.5us slower in the full pipeline (interleaved A/B;
# the extra instructions' semaphore waits clog the PE queue) — keep off
USE_LDW = os.environ.get("KERNEL_LDW", "0") == "1"
# how many tiles ahead of the PV consumer the exp (ACT) pipeline runs
EXP_AHEAD = int(os.environ.get("KERNEL_EXP_AHEAD", "2"))
# S^T PSUM pipeline depth (banks)
STP_BUFS = int(os.environ.get("KERNEL_STP_BUFS", "4"))
# 5-bank rotating PV accumulators (with STP_BUFS=3): the half's first PV
# chunk lands on the bank freed longest ago, removing the evacuation-WAR
# stall at each half boundary
PV5 = os.environ.get("KERNEL_PV5", "0") == "1"
# PSUM evacuation engine split: "mix" alternates DVE/ACT; "dve" puts all
# four copies on DVE (ACT's strict-FIFO queue holds the next half's exps,
# so ACT-side copies delay PV bank release by ~1.5us per half; DVE is
# ahead of PE at the boundary and releases banks in ~0.5us steps)
EVAC = os.environ.get("KERNEL_EVAC", "dve")
# defer each pass's final epilogue past the next pass's S^T prologue
# (cross-pass analog of INTERLEAVE_Q)
XPASS = os.environ.get("KERNEL_XPASS", "1") == "1"
# batch the 4 per-half PV evacuations into one 1MB output DMA
BATCH_OUT = os.environ.get("KERNEL_BATCH_OUT", "0") == "1"
# compute passes per For_i trip in timed full mode (amortizes per-trip cost;
# 8 vs 4 measured -4.7us/pass median in paired interleaved A/B: the For_i
# back edge is an all-engine barrier, so the pipeline drain at each trip
# boundary is paid once per trip)
PASSES = int(os.environ.get("KERNEL_PASSES", "16"))
# pt (exp output) SBUF pool depth
PTP_BUFS = int(os.environ.get("KERNEL_PTP_BUFS", "6"))


def passes_per_iter(mode, loop):
    """Compute passes per For_i iteration (full mode double-buffers inputs
    across iterations with two body emissions per trip)."""
    return 2 if (mode == "full" and loop > 1) else 1

_CACHE = {}
LAST_RESULTS = None


def _build_nc(qk_mode, loop=1, mode="full"):
    import concourse.tile as tile
    import concourse.mybir as mybir
    from concourse import bacc

    f32 = mybir.dt.float32
    bf16 = mybir.dt.bfloat16
    f16 = mybir.dt.float16
    qk_dt = {"f16": f16, "f32r": f32, "f32": f32}[qk_mode]

    nc = bacc.Bacc()

    qk_d = nc.dram_tensor("qk", [Ck, HW], qk_dt, kind="ExternalInput")
    mk_d = nc.dram_tensor("mk", [Ck, M], qk_dt, kind="ExternalInput")
    # host-packed: mv_d[p, t*Cv + c] = mv[t*128 + p, c]
    mv_d = nc.dram_tensor("mv", [128, MT * Cv], bf16, kind="ExternalInput")
    num_d = nc.dram_tensor("num", [HW, Cv], f32, kind="ExternalOutput")
    acc_d = nc.dram_tensor("acc", [128, HW], f32, kind="ExternalOutput")

    Exp = mybir.ActivationFunctionType.Exp
    Copy = mybir.ActivationFunctionType.Copy
    AluOp = mybir.AluOpType

    MKC = 4               # mk DMA chunks
    MVC = 8               # mv DMA chunks

    def emit_dma(nc, tc, big, bset=0, nsfx=""):
        sfx = f"_{bset}"
        nm = sfx + nsfx
        qk_sb = big.tile([Ck, HW], qk_dt, tag="qk_sb" + sfx, name="qk_sb" + nm)
        nc.sync.dma_start(qk_sb[:, :NQS], qk_d[:, :NQS])
        nc.sync.dma_start(qk_sb[:, NQS:], qk_d[:, NQS:])
        mk_sb = big.tile([Ck, M], qk_dt, tag="mk_sb" + sfx, name="mk_sb" + nm)
        mv_sb = big.tile(
            [128, MT, Cv], bf16, tag="mv_sb" + sfx, name="mv_sb" + nm
        )
        # interleave mk / mv chunks so they drain in this order and QK can
        # start after the first mk chunk while mv streams in behind it
        mkw = M // MKC
        mvw = MT // MVC
        for i in range(MVC):
            if i < MKC:
                nc.sync.dma_start(
                    mk_sb[:, i * mkw : (i + 1) * mkw],
                    mk_d[:, i * mkw : (i + 1) * mkw],
                )
            nc.sync.dma_start(
                mv_sb[:, i * mvw : (i + 1) * mvw, :],
                mv_d[:, i * mvw * Cv : (i + 1) * mvw * Cv],
            )
        return qk_sb, mk_sb, mv_sb

    def body(
        nc, tc, big, ptp, accp, outp, stp, pvp, tiles, sfx="", pv_base=0,
        pending=None, defer=False,
    ):
        """Emit one compute pass. `pending` is the previous pass's final
        half-state whose epilogue was deferred; it is emitted after this
        pass's first S^T prologue so the PE starts the new pass's QK MMs
        while DVE/DMA drain the old epilogue. With `defer`, this pass's
        own final epilogue is likewise left to the next body (the caller
        must close the last one before the For_i back edge)."""
        qk_sb, mk_sb, mv_sb = tiles

        class QState:
            def __init__(self, q):
                self.q = q
                self.sts = {}
                self.pts = {}

            def emit_st(self, m):
                st = stp.tile(
                    [128, NQS], f32, tag="st", name=f"st{sfx}_q{self.q}_m{m}"
                )
                wa = mk_sb[:, m * 128 : (m + 1) * 128]
                if USE_LDW:
                    nc.tensor.ldweights(wa)
                nc.tensor.matmul(
                    st[:],
                    wa,
                    qk_sb[:, self.q * NQS : (self.q + 1) * NQS],
                    start=True,
                    stop=True,
                )
                self.sts[m] = st

            def emit_exp(self, m):
                pt = ptp.tile(
                    [128, NQS], bf16, tag="pt", name=f"pt{sfx}_q{self.q}_m{m}"
                )
                nc.scalar.activation(pt[:], self.sts.pop(m)[:], Exp)
                self.pts[m] = pt

            def emit_acc(self, m):
                # VectorE: accumulate exp tiles for the softmax denominator
                if m == 0:
                    nc.vector.tensor_copy(self.acc[:], self.pts[m][:])
                else:
                    nc.vector.tensor_tensor(
                        self.acc[:], self.acc[:], self.pts[m][:], AluOp.add
                    )

            def prologue(self):
                q = self.q
                if PV5:
                    self.pv = [
                        pvp.tile(
                            [128, NQS], f32,
                            tag=f"pvb{(pv_base + q * NCH + i) % 5}",
                            name=f"pv{sfx}_q{q}_{i}",
                        )
                        for i in range(NCH)
                    ]
                else:
                    self.pv = [
                        pvp.tile(
                            [128, NQS], f32, tag=f"pv{i}", name=f"pv{sfx}_q{q}_{i}"
                        )
                        for i in range(NCH)
                    ]
                self.acc = accp.tile([128, NQS], f32, tag="acc", name=f"acc{sfx}_q{q}")
                # software pipeline: PE always has the next S^T ready, ACT
                # runs two tiles ahead of the PV consumers
                self.emit_st(0)
                self.emit_st(1)
                self.emit_exp(0)
                self.emit_st(2)
                self.emit_exp(1)
                self.emit_st(3)
                for e in range(2, EXP_AHEAD):
                    self.emit_exp(e)

            def mloop(self):
                for m in range(MT):
                    ptm = self.pts[m]
                    for nch in range(NCH):
                        wa = ptm[:, nch * 128 : (nch + 1) * 128]
                        if USE_LDW:
                            nc.tensor.ldweights(wa)
                        nc.tensor.matmul(
                            self.pv[nch][:],
                            wa,
                            mv_sb[:, m],
                            start=(m == 0),
                            stop=(m == MT - 1),
                        )
                    if m + EXP_AHEAD < MT:
                        self.emit_exp(m + EXP_AHEAD)
                    self.emit_acc(m)
                    del self.pts[m]
                    if m + 4 < MT:
                        self.emit_st(m + 4)

            def epilogue(self):
                q = self.q
                # raw partition-partial denominators out; host reduces
                nc.sync.dma_start(acc_d[:, q * NQS : (q + 1) * NQS], self.acc[:])
                if BATCH_OUT:
                    o = outp.tile(
                        [128, NCH, Cv], f32, tag="ob", name=f"ob{sfx}_q{q}"
                    )
                    for nch in range(NCH):
                        if nch % 2 == 0:
                            nc.vector.tensor_copy(o[:, nch], self.pv[nch][:])
                        else:
                            nc.scalar.activation(o[:, nch], self.pv[nch][:], Copy)
                    dst = num_d[q * NQS : (q + 1) * NQS, :].rearrange(
                        "(nch p) c -> p nch c", p=128
                    )
                    nc.sync.dma_start(dst, o[:])
                    return
                for nch in range(NCH):
                    o = outp.tile([128, Cv], f32, tag="o", name=f"o{sfx}_q{q}_{nch}")
                    # split PSUM evacuation across DVE and ACT. With PV5 the
                    # next half's chunks 1-3 reuse this half's chunk-0/1/2
                    # banks (chunk 3's bank is the next spare), so release
                    # 0-2 via DVE (ACT is busy with the next half's exps).
                    if EVAC == "dve":
                        use_dve = True
                    elif PV5:
                        use_dve = nch < 3
                    else:
                        use_dve = nch % 2 == 0
                    if use_dve:
                        nc.vector.tensor_copy(o[:], self.pv[nch][:])
                    else:
                        nc.scalar.activation(o[:], self.pv[nch][:], Copy)
                    n0 = q * NQS + nch * 128
                    nc.sync.dma_start(num_d[n0 : n0 + 128, :], o[:])

        # emit the next half's S^T prologue before this half's epilogue so
        # PE isn't FIFO-blocked behind the epilogue's dependencies
        interleave = os.environ.get("KERNEL_INTERLEAVE_Q", "1") == "1"
        states = [QState(q) for q in range(NQ)]
        states[0].prologue()
        if pending is not None:
            pending.epilogue()
        for q in range(NQ):
            states[q].mloop()
            if q + 1 < NQ:
                if interleave:
                    states[q + 1].prologue()
                    states[q].epilogue()
                else:
                    states[q].epilogue()
                    states[q + 1].prologue()
            elif defer and interleave:
                return states[q]
            else:
                states[q].epilogue()
        return None

    with tile.TileContext(nc) as tc:
        with (
            tc.tile_pool(name="big", bufs=1) as big,
            tc.tile_pool(name="ptp", bufs=PTP_BUFS) as ptp,
            tc.tile_pool(name="accp", bufs=3) as accp,
            tc.tile_pool(name="outp", bufs=(2 if BATCH_OUT else 6)) as outp,
            tc.tile_pool(name="stp", bufs=STP_BUFS, space="PSUM") as stp,
            tc.tile_pool(name="pvp", bufs=1, space="PSUM") as pvp,
        ):
            if mode == "mmonly" and loop > 1:
                # pure PE stream: same LDW+MM pair count/shapes as the real
                # kernel, but no ACT/DVE in the loop
                tiles = emit_dma(nc, tc, big)
                qk_sb, mk_sb, mv_sb = tiles
                dummy_pt = big.tile([128, NQS], mybir.dt.bfloat16, tag="dummy_pt", name="dummy_pt")
                nc.vector.memset(dummy_pt[:], 0.001)
                with tc.For_i(0, loop, 1):
                    for q in range(NQ):
                        pv = [
                            pvp.tile([128, NQS], f32, tag=f"pv{i}", name=f"mm_pv_q{q}_{i}")
                            for i in range(NCH)
                        ]
                        for m in range(MT):
                            st = stp.tile([128, NQS], f32, tag="st", name=f"mm_st_q{q}_m{m}")
                            nc.tensor.matmul(
                                st[:],
                                mk_sb[:, m * 128 : (m + 1) * 128],
                                qk_sb[:, q * NQS : (q + 1) * NQS],
                                start=True,
                                stop=True,
                            )
                            for nch in range(NCH):
                                nc.tensor.matmul(
                                    pv[nch][:],
                                    dummy_pt[:, nch * 128 : (nch + 1) * 128],
                                    mv_sb[:, m],
                                    start=(m == 0),
                                    stop=(m == MT - 1),
                                )
                        for nch in range(NCH):
                            o = outp.tile([128, Cv], f32, tag="o", name=f"mm_o_q{q}_{nch}")
                            nc.vector.tensor_copy(o[:], pv[nch][:])
                            nc.sync.dma_start(
                                num_d[(q * NCH + nch) * 128 : (q * NCH + nch + 1) * 128, :],
                                o[:],
                            )
            elif mode == "compute" and loop > 1:
                tiles = emit_dma(nc, tc, big)
                with tc.For_i(0, loop, 1):
                    body(nc, tc, big, ptp, accp, outp, stp, pvp, tiles)
            elif mode == "dma" and loop > 1:
                with tc.For_i(0, loop, 1):
                    emit_dma(nc, tc, big)
                    for j in range(8):
                        o = outp.tile([128, Cv], f32, tag="o", name=f"o_{j}")
                        nc.vector.memset(o[:], float(j))
                        nc.sync.dma_start(num_d[j * 128 : (j + 1) * 128, :], o[:])
            elif loop > 1:
                # full mode, timed: PASSES compute passes per trip over two
                # alternating input buffer sets, so input DMAs stream
                # underneath compute. Back-edge WAR: next trip's set-k DMAs
                # only wait on this trip's second-to-last readers of set k,
                # so they overlap the final passes. Single-buffered, the WAR
                # hazard serializes DMA after compute (~36us exposed/pass).
                with tc.For_i(0, max(loop // PASSES, 1), 1):
                    tiles = {0: emit_dma(nc, tc, big, 0)}
                    if PASSES > 1:
                        tiles[1] = emit_dma(nc, tc, big, 1)
                    pend = None
                    for p in range(PASSES):
                        pend = body(
                            nc, tc, big, ptp, accp, outp, stp, pvp,
                            tiles[p], f"p{p}",
                            pv_base=(p * NQ * NCH) % 5 if PV5 else 0,
                            pending=pend,
                            defer=XPASS and (p + 1 < PASSES),
                        )
                        if p + 2 < PASSES:
                            tiles[p + 2] = emit_dma(
                                nc, tc, big, p % 2, f"_p{p + 2}"
                            )
                    assert pend is None
            else:
                tiles = emit_dma(nc, tc, big)
                body(nc, tc, big, ptp, accp, outp, stp, pvp, tiles)

    nc.finalize()
    return nc


class _null:
    def __enter__(self):
        return None

    def __exit__(self, *a):
        return False


def _get_nc():
    key = ("nc", QK_MODE, LOOP, MODE)
    if key not in _CACHE:
        _CACHE[key] = _build_nc(QK_MODE, LOOP, MODE)
    return _CACHE[key]


def _prep_core_inputs(memory_keys, memory_values, query_key, b):
    np_qk_dt = np.float16 if QK_MODE == "f16" else np.float32
    qk = query_key[b].reshape(Ck, HW).astype(np_qk_dt)
    mk = memory_keys[:, b].transpose(1, 0, 2, 3).astype(np_qk_dt).reshape(Ck, M)
    # (M, Cv) -> tiles of 128 m-rows, partition-major: [128, MT*Cv]
    mv = (
        memory_values[:, b]
        .transpose(0, 2, 3, 1)
        .astype(ml_dtypes.bfloat16)
        .reshape(MT, 128, Cv)
        .transpose(1, 0, 2)
        .reshape(128, MT * Cv)
    )
    mv = np.ascontiguousarray(mv)
    return {"qk": qk, "mk": mk, "mv": mv}


_RUNNER = {}


def _get_runner():
    """Build the sharded PJRT callable once and reuse it."""
    if "r" not in _RUNNER:
        import jax
        from jax.sharding import Mesh, PartitionSpec, NamedSharding
        from jax.experimental.shard_map import shard_map

        import concourse.mybir as mybir
        from concourse import bass2jax
        from concourse.bass2jax import _bass_exec_p, install_neuronx_cc_hook

        nc = _get_nc()
        install_neuronx_cc_hook()
        pname = nc.partition_id_tensor.name if nc.partition_id_tensor else None
        in_names, out_names, out_avals = [], [], []
        for alloc in nc.m.functions[0].allocations:
            if not isinstance(alloc, mybir.MemoryLocationSet):
                continue
            name = alloc.memorylocations[0].name
            if alloc.kind == "ExternalInput":
                if name != pname:
                    in_names.append(name)
            elif alloc.kind == "ExternalOutput":
                out_names.append(name)
                out_avals.append(
                    jax.core.ShapedArray(
                        tuple(alloc.tensor_shape), mybir.dt.np(alloc.dtype)
                    )
                )
        n_params = len(in_names)
        all_in = list(in_names) + list(out_names) + ([pname] if pname else [])

        def _body(*args):
            operands = list(args)
            if pname is not None:
                operands.append(bass2jax.partition_id_tensor())
            return tuple(
                _bass_exec_p.bind(
                    *operands,
                    out_avals=tuple(out_avals),
                    in_names=tuple(all_in),
                    out_names=tuple(out_names),
                    lowering_input_output_aliases=(),
                    sim_require_finite=True,
                    sim_require_nnan=True,
                    nc=nc,
                )
            )

        mesh = Mesh(np.asarray(jax.devices()[:N_CORES]), ("core",))
        n_outs = len(out_names)
        sharded = jax.jit(
            shard_map(
                _body,
                mesh=mesh,
                in_specs=(PartitionSpec("core"),) * (n_params + n_outs),
                out_specs=(PartitionSpec("core"),) * n_outs,
                check_rep=False,
            ),
            keep_unused=True,
        )
        sh = NamedSharding(mesh, PartitionSpec("core"))
        zeros = [
            jax.device_put(
                np.zeros((N_CORES * a.shape[0], *a.shape[1:]), a.dtype), sh
            )
            for a in out_avals
        ]
        _RUNNER["r"] = (sharded, sh, in_names, out_names, zeros)
    return _RUNNER["r"]


def _finish(num_full, acc_full, query_value):
    """Host-side softmax normalization + output assembly."""
    mems = []
    for bb in range(N_CORES):
        den = acc_full[bb].astype(np.float64).sum(axis=0)  # (HW,)
        mem = (num_full[bb].astype(np.float64) / den[:, None]).T  # (Cv, HW)
        mems.append(mem.astype(np.float32).reshape(Cv, H, W))
    return np.concatenate([query_value, np.stack(mems)], axis=1)


def kernel(memory_keys, memory_values, query_key, query_value):
    global LAST_RESULTS
    memory_keys = np.asarray(memory_keys, dtype=np.float32)
    memory_values = np.asarray(memory_values, dtype=np.float32)
    query_key = np.asarray(query_key, dtype=np.float32)
    query_value = np.asarray(query_value, dtype=np.float32)

    in_maps = [
        _prep_core_inputs(memory_keys, memory_values, query_key, b)
        for b in range(N_CORES)
    ]
    try:
        import jax

        sharded, sh, in_names, out_names, zeros = _get_runner()
        dev_in = [
            jax.device_put(
                np.concatenate([in_maps[c][n] for c in range(N_CORES)], 0), sh
            )
            for n in in_names
        ]
        outs = sharded(*dev_in, *zeros)
        res = {n: np.asarray(o) for n, o in zip(out_names, outs)}
        num_full = res["num"].reshape(N_CORES, HW, Cv)
        acc_full = res["acc"].reshape(N_CORES, 128, HW)
    except Exception:
        from concourse.bass_utils import run_bass_kernel_spmd

        res = run_bass_kernel_spmd(
            _get_nc(), in_maps, core_ids=list(range(N_CORES))
        )
        LAST_RESULTS = res
        num_full = np.stack([res.results[b]["num"] for b in range(N_CORES)])
        acc_full = np.stack([res.results[b]["acc"] for b in range(N_CORES)])

    return _finish(num_full, acc_full, query_value)

